# revision 14
# baseline (speedup 1.0000x reference)
"""Trainium2 Bass kernel for per-time-slice spatial self-attention + 1x1 conv.

Math per (b, t) slice (16 slices total):
    x      = x_in[b, :, t]          reshaped [C=64, P=2304]
    theta  = theta_w[t] @ x         [32, P]
    phi    = phi_w[t] @ x           [32, P]
    S      = theta.T @ phi / sqrt(32)          [P, P]
    A      = softmax(S, axis=-1)
    f      = x @ A.T  (f[c,p] = sum_q A[p,q] x[c,q])
    out    = out_w @ f + x

Sharding: the 16 slices are independent -> 2 slices per NeuronCore, no
collectives. Host precomputes the cheap channel projections (theta, phi,
v = out_w @ x) and packs layouts; the device runs the O(P^2) attention core:

  per p-chunk (4x512 + 256) accumulating in PSUM over 18 q-tiles of 128:
    scoresT[q, p] = sum_c phi[c, q] theta[c, p]   (PE, K=32)
    E ~= exp(scoresT / sqrt(32))                  (VectorE, one fused
        tensor_scalar: i16 = int16(s * 128*log2e*scale + 16256); the i16
        bit pattern bitcast as bf16 IS the Schraudolph base-2 exp
        approximation. ScalarE's activation LUT path measures ~7x slower
        than one DVE op on this part, and the softmax normalization
        cancels the bulk of the correlated approximation error:
        end-to-end max rel err ~2e-3 vs 8e-4 with exact exp.)
    val[m, p] += vte[q, m]^T E[q, p]              (PE, m: 64 v-channels + ones
                                                   column -> softmax denom)
  epilogue: one DVE copy val -> SBUF per chunk; one [65, 2304] DMA per
  slice (output DMAs ride the qAct HWDGE ring, inputs ride qSP, so the
  two streams don't serialize). The final normalization (row 64 divide)
  and the +x residual are done on the host after the gather.

exp skips max-subtraction (scores ~ N(0,1), max |s| ~ 6; safe in this
fixed-point window: t = 128*z + 16256 stays within int16 for |z| < 127).
"""

import os
import sys

for _p in ("/opt/trn_rl_repo", "/root/.axon_site/_ro/trn_rl_repo"):
    if os.path.isdir(_p) and _p not in sys.path:
        sys.path.append(_p)

# The axon NTFF profiling hook (antenv.axon_hooks) is absent in this
# container; make sure run_bass_kernel_spmd never takes the trace path.
os.environ["BASS_NEVER_TRACE"] = "1"

import numpy as np
from contextlib import ExitStack

import concourse.bass as bass
import concourse.tile as tile
from concourse import bacc, mybir
from concourse.bass_utils import run_bass_kernel_spmd

B, C, T, H, W = 2, 64, 8, 48, 48
C2 = 32
P = H * W                      # 2304
N_CORES = 8
S_PER_CORE = (B * T) // N_CORES  # 2 slices per core
QT = P // 128                  # 18 q-tiles of 128
GSZ = 3                        # q-tiles per exp group (3 PSUM banks)
P_CHUNKS = [(0, 512), (512, 512), (1024, 512), (1536, 512), (2048, 256)]
SCALE = 1.0 / np.sqrt(np.float32(C2))
# Schraudolph constants: int16(z * 128*log2(e)*SCALE + 16256) bitcast bf16
SCH_A = float(128.0 * 1.4426950408889634 * SCALE)
SCH_B = 16256.0

F32 = mybir.dt.float32
BF16 = mybir.dt.bfloat16
I16 = mybir.dt.int16
ALU = mybir.AluOpType

_CACHE = {}


def build_nc(repeat=1):
    """Build the per-core Bass program (SPMD: same NEFF on all 8 cores).

    repeat > 1 re-runs the whole computation; used only for timing (the
    extra passes recompute and overwrite the same outputs).
    """
    nc = bacc.Bacc("TRN2", target_bir_lowering=False, debug=False,
                   num_devices=N_CORES)
    # both slices' theta||phi on 32 partitions: [32, s*4608 + (th | ph)]
    thph_d = nc.dram_tensor("thph", [C2, S_PER_CORE * 2 * P], BF16,
                            kind="ExternalInput").ap()
    vte_d = nc.dram_tensor("vte", [128, S_PER_CORE * QT * (C + 1)], BF16,
                           kind="ExternalInput").ap()
    y_d = nc.dram_tensor("y", [S_PER_CORE, C + 1, P], F32,
                         kind="ExternalOutput").ap()

    with tile.TileContext(nc) as tc, ExitStack() as ctx:
        ins = ctx.enter_context(tc.tile_pool(name="ins", bufs=2))
        epool = ctx.enter_context(tc.tile_pool(name="epool", bufs=3))
        scp = ctx.enter_context(tc.tile_pool(name="scp", bufs=2, space="PSUM"))
        valp = ctx.enter_context(tc.tile_pool(name="valp", bufs=2,
                                              space="PSUM"))
        epi = ctx.enter_context(tc.tile_pool(name="epi", bufs=2))

        for r in range(repeat):
            thph_sb = ins.tile([C2, S_PER_CORE * 2 * P], BF16, tag="thph")
            nc.sync.dma_start(out=thph_sb, in_=thph_d)
            vte_sb = ins.tile([128, S_PER_CORE, QT, C + 1], BF16, tag="vte")
            nc.sync.dma_start(out=vte_sb, in_=vte_d.rearrange(
                "p (s q m) -> p s q m", s=S_PER_CORE, q=QT))

            for s in range(S_PER_CORE):
                th0 = s * 2 * P          # theta cols for this slice
                ph0 = s * 2 * P + P      # phi cols
                o_slice = epi.tile([C + 1, P], F32, tag="oslice")
                for (off, w) in P_CHUNKS:
                    val = valp.tile([C + 1, w], F32, tag="val")

                    def emit_val(e_sb, g):
                        for j in range(GSZ):
                            qt = g * GSZ + j
                            # val[m, p] += sum_q vte[q, m] * E[q, p]
                            nc.tensor.matmul(
                                out=val,
                                lhsT=vte_sb[:, s, qt, :],
                                rhs=e_sb[:, j, :].bitcast(BF16),
                                start=(qt == 0), stop=(qt == QT - 1),
                            )

                    # software pipeline: the val matmuls of group g-1 are
                    # emitted AFTER the scores of group g, so the PE queue
                    # always holds work that does not depend on the exp of
                    # the group currently on the DVE (the PE engine queue is
                    # strict FIFO - without this, every group serializes
                    # into a PE -> DVE -> PE round-trip). NOTE: carrying the
                    # pending val matmuls across the chunk boundary (so the
                    # per-chunk copy lands between exp ops on the DVE) was
                    # measured 2x SLOWER - it just moves the head-of-line
                    # blocking onto the DVE queue. Keep the drain per-chunk.
                    pending = None
                    for g in range(QT // GSZ):
                        sc = scp.tile([128, GSZ, w], F32, tag="sc")
                        for j in range(GSZ):
                            qt = g * GSZ + j
                            # scoresT[q, p] = sum_c phi[c, q] * theta[c, p]
                            nc.tensor.matmul(
                                out=sc[:, j, :],
                                lhsT=thph_sb[:, ph0 + qt * 128:
                                             ph0 + (qt + 1) * 128],
                                rhs=thph_sb[:, th0 + off:th0 + off + w],
                                start=True, stop=True,
                            )
                        if pending is not None:
                            emit_val(*pending)
                        # E = schraudolph-exp(sc * SCALE) in bf16 bits
                        e_sb = epool.tile([128, GSZ, w], I16, tag="E")
                        nc.vector.tensor_scalar(
                            out=e_sb, in0=sc, scalar1=SCH_A, scalar2=SCH_B,
                            op0=ALU.mult, op1=ALU.add)
                        pending = (e_sb, g)
                    emit_val(*pending)
                    nc.vector.tensor_copy(out=o_slice[:, off:off + w],
                                          in_=val)
                # output DMA on the qAct ring (inputs ride qSP)
                nc.scalar.dma_start(out=y_d[s], in_=o_slice)

    nc.compile()
    return nc


def host_prep(x_in, theta_w, phi_w, out_w):
    """Per-core input maps: channel projections + device layouts (numpy)."""
    import ml_dtypes
    mmdt = np.dtype(ml_dtypes.bfloat16)
    x_in = np.ascontiguousarray(x_in, dtype=np.float32)
    theta_w = np.asarray(theta_w, dtype=np.float32)
    phi_w = np.asarray(phi_w, dtype=np.float32)
    out_w = np.asarray(out_w, dtype=np.float32)

    x = np.transpose(x_in, (0, 2, 1, 3, 4)).reshape(B, T, C, P)

    in_maps = []
    for k in range(N_CORES):
        thph = np.empty((C2, S_PER_CORE * 2 * P), mmdt)
        vte = np.empty((128, S_PER_CORE * QT * (C + 1)), mmdt)
        vte_v = vte.reshape(128, S_PER_CORE, QT, C + 1)
        for s in range(S_PER_CORE):
            g = k * S_PER_CORE + s
            b, t = divmod(g, T)
            xslice = x[b, t]                      # [C, P]
            thph[:, s * 2 * P:s * 2 * P + P] = theta_w[t] @ xslice
            thph[:, s * 2 * P + P:(s + 1) * 2 * P] = phi_w[t] @ xslice
            v = out_w @ xslice                    # [64, P]
            vt = np.empty((QT, 128, C + 1), mmdt)
            vt[:, :, :C] = v.T.reshape(QT, 128, C)
            vt[:, :, C] = 1.0                     # softmax-denominator column
            vte_v[:, s] = np.transpose(vt, (1, 0, 2))
        in_maps.append({"thph": thph, "vte": vte})
    return in_maps


def assemble(results, x_in):
    out = np.empty((B, C, T, H, W), np.float32)
    for k in range(N_CORES):
        y = results[k]["y"]  # [S_PER_CORE, C+1, P]: numerator rows + denom
        for s in range(S_PER_CORE):
            g = k * S_PER_CORE + s
            b, t = divmod(g, T)
            yn = y[s, :C] / y[s, C:C + 1]
            out[b, :, t] = yn.reshape(C, H, W) + x_in[b, :, t]
    return out


def kernel(x_in, theta_w, phi_w, out_w):
    if "nc" not in _CACHE:
        _CACHE["nc"] = build_nc()
    nc = _CACHE["nc"]
    in_maps = host_prep(x_in, theta_w, phi_w, out_w)
    res = run_bass_kernel_spmd(nc, in_maps, core_ids=list(range(N_CORES)))
    return assemble(res.results, np.asarray(x_in, dtype=np.float32))


# revision 17
# speedup vs baseline: 1.6107x; 1.6107x over previous
"""Trainium2 Bass kernel for per-time-slice spatial self-attention + 1x1 conv.

Math per (b, t) slice (16 slices total):
    x      = x_in[b, :, t]          reshaped [C=64, P=2304]
    theta  = theta_w[t] @ x         [32, P]
    phi    = phi_w[t] @ x           [32, P]
    S      = theta.T @ phi / sqrt(32)          [P, P]
    A      = softmax(S, axis=-1)
    f      = x @ A.T  (f[c,p] = sum_q A[p,q] x[c,q])
    out    = out_w @ f + x

Sharding: the 16 slices are independent -> 2 slices per NeuronCore, no
collectives. Host precomputes the cheap channel projections (theta, phi,
v = out_w @ x) and packs layouts; the device runs the O(P^2) attention core:

  per p-chunk (4x512 + 256) accumulating in PSUM over 18 q-tiles of 128:
    scoresT[q, p] = sum_c phi[c, q] theta[c, p]   (PE, K=32)
    E ~= exp(scoresT / sqrt(32))                  (VectorE, one fused
        tensor_scalar: i16 = int16(s * 128*log2e*scale + 16256); the i16
        bit pattern bitcast as bf16 IS the Schraudolph base-2 exp
        approximation. ScalarE's activation LUT path measures ~7x slower
        than one DVE op on this part, and the softmax normalization
        cancels the bulk of the correlated approximation error:
        end-to-end max rel err ~2e-3 vs 8e-4 with exact exp.)
    val[m, p] += vte[q, m]^T E[q, p]              (PE, m: 64 v-channels + ones
                                                   column -> softmax denom)
  epilogue: one DVE copy val -> SBUF per chunk; one [65, 2304] DMA per
  slice (output DMAs ride the qAct HWDGE ring, inputs ride qSP, so the
  two streams don't serialize). The final normalization (row 64 divide)
  and the +x residual are done on the host after the gather.

exp skips max-subtraction (scores ~ N(0,1), max |s| ~ 6; safe in this
fixed-point window: t = 128*z + 16256 stays within int16 for |z| < 127).
"""

import os
import sys

for _p in ("/opt/trn_rl_repo", "/root/.axon_site/_ro/trn_rl_repo"):
    if os.path.isdir(_p) and _p not in sys.path:
        sys.path.append(_p)

# The axon NTFF profiling hook (antenv.axon_hooks) is absent in this
# container; make sure run_bass_kernel_spmd never takes the trace path.
os.environ["BASS_NEVER_TRACE"] = "1"

import numpy as np
from collections import deque
from contextlib import ExitStack

import concourse.bass as bass
import concourse.tile as tile
from concourse import bacc, mybir
from concourse.bass_utils import run_bass_kernel_spmd

B, C, T, H, W = 2, 64, 8, 48, 48
C2 = 32
P = H * W                      # 2304
N_CORES = 8
S_PER_CORE = (B * T) // N_CORES  # 2 slices per core
QT = P // 128                  # 18 q-tiles of 128
GSZ = 3                        # q-tiles per exp group (3 PSUM banks)
P_CHUNKS = [(0, 512), (512, 512), (1024, 512), (1536, 512), (2048, 256)]
SCALE = 1.0 / np.sqrt(np.float32(C2))
# Schraudolph constants: int16(z * 128*log2(e)*SCALE + 16256) bitcast bf16
SCH_A = float(128.0 * 1.4426950408889634 * SCALE)
SCH_B = 16256.0

F32 = mybir.dt.float32
BF16 = mybir.dt.bfloat16
I16 = mybir.dt.int16
ALU = mybir.AluOpType

_CACHE = {}


def build_nc(repeat=1):
    """Build the per-core Bass program (SPMD: same NEFF on all 8 cores).

    repeat > 1 re-runs the whole computation; used only for timing (the
    extra passes recompute and overwrite the same outputs).
    """
    nc = bacc.Bacc("TRN2", target_bir_lowering=False, debug=False,
                   num_devices=N_CORES)
    # both slices' theta||phi on 32 partitions: [32, s*4608 + (th | ph)]
    thph_d = nc.dram_tensor("thph", [C2, S_PER_CORE * 2 * P], BF16,
                            kind="ExternalInput").ap()
    vte_d = nc.dram_tensor("vte", [128, S_PER_CORE * QT * (C + 1)], BF16,
                           kind="ExternalInput").ap()
    y_d = nc.dram_tensor("y", [S_PER_CORE, C + 1, P], F32,
                         kind="ExternalOutput").ap()

    with tile.TileContext(nc) as tc, ExitStack() as ctx:
        ins = ctx.enter_context(tc.tile_pool(name="ins", bufs=2))
        epool = ctx.enter_context(tc.tile_pool(name="epool", bufs=4))
        scp = ctx.enter_context(tc.tile_pool(name="scp", bufs=2, space="PSUM"))
        valp = ctx.enter_context(tc.tile_pool(name="valp", bufs=2,
                                              space="PSUM"))
        epi = ctx.enter_context(tc.tile_pool(name="epi", bufs=2))

        for r in range(repeat):
            thph_sb = ins.tile([C2, S_PER_CORE * 2 * P], BF16, tag="thph")
            nc.sync.dma_start(out=thph_sb, in_=thph_d)
            vte_sb = ins.tile([128, S_PER_CORE, QT, C + 1], BF16, tag="vte")
            nc.sync.dma_start(out=vte_sb, in_=vte_d.rearrange(
                "p (s q m) -> p s q m", s=S_PER_CORE, q=QT))

            for s in range(S_PER_CORE):
                th0 = s * 2 * P          # theta cols for this slice
                ph0 = s * 2 * P + P      # phi cols
                o_slice = epi.tile([C + 1, P], F32, tag="oslice")
                for (off, w) in P_CHUNKS:
                    val = valp.tile([C + 1, w], F32, tag="val")

                    def emit_val(e_sb, g):
                        for j in range(GSZ):
                            qt = g * GSZ + j
                            # val[m, p] += sum_q vte[q, m] * E[q, p]
                            nc.tensor.matmul(
                                out=val,
                                lhsT=vte_sb[:, s, qt, :],
                                rhs=e_sb[:, j, :].bitcast(BF16),
                                start=(qt == 0), stop=(qt == QT - 1),
                            )

                    # software pipeline: the val matmuls of group g-2 are
                    # emitted AFTER the scores of group g, so the PE queue
                    # always holds work that does not depend on the exp of
                    # the group currently on the DVE (the PE engine queue is
                    # strict FIFO - without this, every group serializes
                    # into a PE -> DVE -> PE round-trip). NOTE: carrying the
                    # pending val matmuls across the chunk boundary (so the
                    # per-chunk copy lands between exp ops on the DVE) was
                    # measured 2x SLOWER - it just moves the head-of-line
                    # blocking onto the DVE queue. Keep the drain per-chunk.
                    pend = deque()
                    for g in range(QT // GSZ):
                        sc = scp.tile([128, GSZ, w], F32, tag="sc")
                        for j in range(GSZ):
                            qt = g * GSZ + j
                            # scoresT[q, p] = sum_c phi[c, q] * theta[c, p]
                            nc.tensor.matmul(
                                out=sc[:, j, :],
                                lhsT=thph_sb[:, ph0 + qt * 128:
                                             ph0 + (qt + 1) * 128],
                                rhs=thph_sb[:, th0 + off:th0 + off + w],
                                start=True, stop=True,
                            )
                        if len(pend) == 2:
                            emit_val(*pend.popleft())
                        # E = schraudolph-exp(sc * SCALE) in bf16 bits
                        e_sb = epool.tile([128, GSZ, w], I16, tag="E")
                        nc.vector.tensor_scalar(
                            out=e_sb, in0=sc, scalar1=SCH_A, scalar2=SCH_B,
                            op0=ALU.mult, op1=ALU.add)
                        pend.append((e_sb, g))
                    while pend:
                        emit_val(*pend.popleft())
                    # val -> SBUF on the (otherwise idle) ScalarE: a DVE copy
                    # here would sit between exp ops on the DVE FIFO and
                    # stall them on the PE's last val matmul at every chunk
                    # boundary (same head-of-line pattern as above).
                    nc.scalar.copy(out=o_slice[:, off:off + w], in_=val)
                # output DMA on the qAct ring (inputs ride qSP)
                nc.scalar.dma_start(out=y_d[s], in_=o_slice)

    nc.compile()
    return nc


def host_prep(x_in, theta_w, phi_w, out_w):
    """Per-core input maps: channel projections + device layouts (numpy)."""
    import ml_dtypes
    mmdt = np.dtype(ml_dtypes.bfloat16)
    x_in = np.ascontiguousarray(x_in, dtype=np.float32)
    theta_w = np.asarray(theta_w, dtype=np.float32)
    phi_w = np.asarray(phi_w, dtype=np.float32)
    out_w = np.asarray(out_w, dtype=np.float32)

    x = np.transpose(x_in, (0, 2, 1, 3, 4)).reshape(B, T, C, P)

    in_maps = []
    for k in range(N_CORES):
        thph = np.empty((C2, S_PER_CORE * 2 * P), mmdt)
        vte = np.empty((128, S_PER_CORE * QT * (C + 1)), mmdt)
        vte_v = vte.reshape(128, S_PER_CORE, QT, C + 1)
        for s in range(S_PER_CORE):
            g = k * S_PER_CORE + s
            b, t = divmod(g, T)
            xslice = x[b, t]                      # [C, P]
            thph[:, s * 2 * P:s * 2 * P + P] = theta_w[t] @ xslice
            thph[:, s * 2 * P + P:(s + 1) * 2 * P] = phi_w[t] @ xslice
            v = out_w @ xslice                    # [64, P]
            vt = np.empty((QT, 128, C + 1), mmdt)
            vt[:, :, :C] = v.T.reshape(QT, 128, C)
            vt[:, :, C] = 1.0                     # softmax-denominator column
            vte_v[:, s] = np.transpose(vt, (1, 0, 2))
        in_maps.append({"thph": thph, "vte": vte})
    return in_maps


def assemble(results, x_in):
    out = np.empty((B, C, T, H, W), np.float32)
    for k in range(N_CORES):
        y = results[k]["y"]  # [S_PER_CORE, C+1, P]: numerator rows + denom
        for s in range(S_PER_CORE):
            g = k * S_PER_CORE + s
            b, t = divmod(g, T)
            yn = y[s, :C] / y[s, C:C + 1]
            out[b, :, t] = yn.reshape(C, H, W) + x_in[b, :, t]
    return out


def kernel(x_in, theta_w, phi_w, out_w):
    if "nc" not in _CACHE:
        _CACHE["nc"] = build_nc()
    nc = _CACHE["nc"]
    in_maps = host_prep(x_in, theta_w, phi_w, out_w)
    res = run_bass_kernel_spmd(nc, in_maps, core_ids=list(range(N_CORES)))
    return assemble(res.results, np.asarray(x_in, dtype=np.float32))
